# revision 1
# baseline (speedup 1.0000x reference)
"""Child-Sum TreeLSTM (perfect binary tree, depth 14) on 8 Trainium2 NeuronCores.

Strategy
--------
Heap-order contiguous node sharding: core k owns nodes [k*n/8, (k+1)*n/8) of
every level lvl >= 3 (n = 2^lvl). Children of a core's node range at level lvl
are exactly its node range at level lvl+1, so levels 13..3 run with zero
cross-core communication (stage A, SPMD on 8 cores). Levels 2..0 (7 nodes) run
on core 0 (stage B) from the gathered 8 level-3 (h, c) states.

Within a core, state is kept transposed: [mem_dim(1024) partitions x n nodes],
stored as 8 per-M-tile SBUF tiles of [128, n]. Per level, each gate
pre-activation accumulates in PSUM: 8 K-chunk matmuls against Wh plus one
identity-matmul that adds the precomputed x-projection (which already includes
both biases). All matmul inputs are bf16 (fp32 PSUM accumulation); stored
states are bf16. x-projections for all 2047 nodes/core are computed upfront in
one dense pass (leaf level fused into it) and staged in DRAM.
"""

import numpy as np
import ml_dtypes
from contextlib import ExitStack

import concourse.bass as bass
import concourse.tile as tile
from concourse import bacc, mybir
from concourse.bass_utils import run_bass_kernel_spmd

BF16 = ml_dtypes.bfloat16
P = 128
MEM = 1024
IN = 300
DEPTH = 14
NCORE = 8
MT = MEM // P  # 8 M-tiles of the mem dim

NX = 2047  # per-core nodes, levels 13..3
NTOP = 7  # top nodes (levels 2..0), xproj computed by stage A
NXT = NX + NTOP  # 2054

GATES = "ifou"  # gate index order everywhere
ACT_FN = {
    "i": mybir.ActivationFunctionType.Sigmoid,
    "f": mybir.ActivationFunctionType.Sigmoid,
    "o": mybir.ActivationFunctionType.Sigmoid,
    "u": mybir.ActivationFunctionType.Tanh,
}

# per-core column offset of level lvl within the node axis (levels 13..3)
OFF = {13: 0}
for _lvl in range(12, 2, -1):
    OFF[_lvl] = OFF[_lvl + 1] + (2 ** (_lvl + 1)) // NCORE

# xproj col chunks: two leaf chunks (fused, i/o/u only), two mid chunks
# (written to DRAM, all gates), one top chunk (written to xtop out, all gates)
LEAF_CHUNKS = [(0, 512), (512, 512)]
MID_CHUNKS = [(1024, 512), (1536, 511)]
TOP_CHUNK = (NX, NTOP)

F32 = mybir.dt.float32
BF = mybir.dt.bfloat16


def _emit_xproj_and_leaf(nc, wh, xt, bias, wxT_d, xproj, xtop_d, pools):
    """x-projection pass (all 2054 cols, bias folded in) with the leaf level
    (lvl 13) fused. Returns per-M-tile leaf state tiles h13, c13 (bf16)."""
    wxp, xpp, gp, hp, psum = pools
    h13, c13 = [], []
    for m in range(MT):
        h13m = hp.tile([P, 1024], BF, tag=f"h13_{m}", name=f"h13_{m}")
        c13m = hp.tile([P, 1024], BF, tag=f"c13_{m}", name=f"c13_{m}")
        leaf = {}  # (g, cci) -> gate tile
        for g in "iouf":
            gi = GATES.index(g)
            wx = []
            for kx in range(3):
                w = wxp.tile([P, P], BF, tag="wx", bufs=6, name=f"wx_{m}_{g}{kx}")
                nc.sync.dma_start(w[:], wxT_d[:, (gi * 3 + kx) * MEM + m * P : (gi * 3 + kx) * MEM + (m + 1) * P])
                wx.append(w)
            chunks = (LEAF_CHUNKS if g != "f" else []) + MID_CHUNKS + [TOP_CHUNK]
            for c0, ncol in chunks:
                ps = psum.tile([P, ncol], F32, tag="ps", bufs=8, name=f"ps_{m}_{g}{c0}")
                for kx in range(3):
                    nc.tensor.matmul(
                        ps[:],
                        wx[kx][:],
                        xt[:, kx * NXT + c0 : kx * NXT + c0 + ncol],
                        start=(kx == 0),
                        stop=(kx == 2),
                    )
                if (c0, ncol) in LEAF_CHUNKS:
                    # leaf gate: activation straight from PSUM (bias via ACT)
                    tg = gp.tile([P, ncol], F32, tag="g", bufs=12, name=f"lf_{m}_{g}{c0}")
                    nc.scalar.activation(tg[:], ps[:], ACT_FN[g], bias=bias[:, gi * MT + m : gi * MT + m + 1])
                    leaf[(g, c0)] = tg
                else:
                    xp = xpp.tile([P, ncol], BF, tag="xp", bufs=8, name=f"xp_{m}_{g}{c0}")
                    nc.scalar.activation(
                        xp[:],
                        ps[:],
                        mybir.ActivationFunctionType.Identity,
                        bias=bias[:, gi * MT + m : gi * MT + m + 1],
                    )
                    if (c0, ncol) == TOP_CHUNK:
                        nc.sync.dma_start(xtop_d[:, (gi * MT + m) * NTOP : (gi * MT + m + 1) * NTOP], xp[:])
                    else:
                        nc.sync.dma_start(xproj[:, (gi * MT + m) * NXT + c0 : (gi * MT + m) * NXT + c0 + ncol], xp[:])
        # leaf combine: c = i*u ; h = o*tanh(c)
        for c0, ncol in LEAF_CHUNKS:
            cs = c13m[:, c0 : c0 + ncol]
            nc.vector.tensor_mul(cs, leaf[("i", c0)][:], leaf[("u", c0)][:])
            th = gp.tile([P, ncol], F32, tag="g", bufs=12, name=f"th13_{m}_{c0}")
            nc.scalar.activation(th[:], cs, mybir.ActivationFunctionType.Tanh)
            nc.vector.tensor_mul(h13m[:, c0 : c0 + ncol], leaf[("o", c0)][:], th[:])
        h13.append(h13m)
        c13.append(c13m)
    return h13, c13


def _emit_level(nc, wh, ident, h_ch, c_ch, n, xp_of, par, pools, lvl):
    """One non-leaf level. h_ch/c_ch: lists of 8 child tiles [128, 2n] (bf16).
    xp_of(gi, m) -> SBUF AP [128, n] holding xproj+bias for that gate/M-tile.
    Returns (h_new, c_new): lists of 8 tiles [128, n] bf16."""
    wxp, xpp, gp, hp, psum = pools
    n2 = 2 * n

    def wh_ap(gi, k, m):
        return wh[:, (gi * MT + k) * MEM + m * P : (gi * MT + k) * MEM + (m + 1) * P]

    # hsum per K-chunk
    hs = []
    for k in range(MT):
        t = hp.tile([P, n], BF, tag=f"hs{par}_{k}", name=f"hs_{lvl}_{k}")
        nc.vector.tensor_add(t[:], h_ch[k][:, 0:n2:2], h_ch[k][:, 1:n2:2])
        hs.append(t)

    h_new, c_new = [], []
    for m in range(MT):
        pre = {}
        for g in "iou":
            gi = GATES.index(g)
            ps = psum.tile([P, n], F32, tag="ps", bufs=8, name=f"ps_{lvl}_{m}{g}")
            for k in range(MT):
                nc.tensor.matmul(ps[:], wh_ap(gi, k, m), hs[k][:], start=(k == 0), stop=False)
            nc.tensor.matmul(ps[:], ident[:], xp_of(gi, m), start=False, stop=True)
            tg = gp.tile([P, n], F32, tag="g", bufs=12, name=f"t{g}_{lvl}_{m}")
            nc.scalar.activation(tg[:], ps[:], ACT_FN[g])
            pre[g] = tg
        # f gate per child
        gi = GATES.index("f")
        psL = psum.tile([P, n], F32, tag="ps", bufs=8, name=f"psL_{lvl}_{m}")
        psR = psum.tile([P, n], F32, tag="ps", bufs=8, name=f"psR_{lvl}_{m}")
        for k in range(MT):
            w = wh_ap(gi, k, m)
            nc.tensor.matmul(psL[:], w, h_ch[k][:, 0:n2:2], start=(k == 0), stop=False)
            nc.tensor.matmul(psR[:], w, h_ch[k][:, 1:n2:2], start=(k == 0), stop=False)
        xpf = xp_of(gi, m)
        nc.tensor.matmul(psL[:], ident[:], xpf, start=False, stop=True)
        nc.tensor.matmul(psR[:], ident[:], xpf, start=False, stop=True)
        fL = gp.tile([P, n], F32, tag="g", bufs=12, name=f"fL_{lvl}_{m}")
        nc.scalar.activation(fL[:], psL[:], ACT_FN["f"])
        fR = gp.tile([P, n], F32, tag="g", bufs=12, name=f"fR_{lvl}_{m}")
        nc.scalar.activation(fR[:], psR[:], ACT_FN["f"])
        # fc = fL*cL + fR*cR
        t1 = gp.tile([P, n], F32, tag="g", bufs=12, name=f"t1_{lvl}_{m}")
        nc.vector.tensor_mul(t1[:], fL[:], c_ch[m][:, 0:n2:2])
        t2 = gp.tile([P, n], F32, tag="g", bufs=12, name=f"t2_{lvl}_{m}")
        nc.vector.tensor_mul(t2[:], fR[:], c_ch[m][:, 1:n2:2])
        fc = gp.tile([P, n], F32, tag="g", bufs=12, name=f"fc_{lvl}_{m}")
        nc.vector.tensor_add(fc[:], t1[:], t2[:])
        # c = i*u + fc ; h = o*tanh(c)
        tiu = gp.tile([P, n], F32, tag="g", bufs=12, name=f"tiu_{lvl}_{m}")
        nc.vector.tensor_mul(tiu[:], pre["i"][:], pre["u"][:])
        cm = hp.tile([P, n], BF, tag=f"c{par}_{m}", name=f"c_{lvl}_{m}")
        nc.vector.tensor_add(cm[:], tiu[:], fc[:])
        th = gp.tile([P, n], F32, tag="g", bufs=12, name=f"th_{lvl}_{m}")
        nc.scalar.activation(th[:], cm[:], mybir.ActivationFunctionType.Tanh)
        hm = hp.tile([P, n], BF, tag=f"h{par}_{m}", name=f"h_{lvl}_{m}")
        nc.vector.tensor_mul(hm[:], pre["o"][:], th[:])
        h_new.append(hm)
        c_new.append(cm)
    return h_new, c_new


def build_stage_a():
    nc = bacc.Bacc("TRN2", target_bir_lowering=False, debug=False, num_devices=NCORE)
    xT_d = nc.dram_tensor("xT", [P, 3 * NXT], BF, kind="ExternalInput").ap()
    wxT_d = nc.dram_tensor("wxT", [P, 4 * 3 * MEM], BF, kind="ExternalInput").ap()
    whT_d = nc.dram_tensor("whT", [P, 4 * MT * MEM], BF, kind="ExternalInput").ap()
    bias_d = nc.dram_tensor("bias", [P, 32], F32, kind="ExternalInput").ap()
    ident_d = nc.dram_tensor("ident", [P, P], BF, kind="ExternalInput").ap()
    h3c3_d = nc.dram_tensor("h3c3", [P, 16], F32, kind="ExternalOutput").ap()
    xtop_d = nc.dram_tensor("xtop", [P, 32 * NTOP], BF, kind="ExternalOutput").ap()

    with tile.TileContext(nc) as tc, ExitStack() as ctx:
        const = ctx.enter_context(tc.tile_pool(name="const", bufs=1))
        wxp = ctx.enter_context(tc.tile_pool(name="wxp", bufs=6))
        xpp = ctx.enter_context(tc.tile_pool(name="xpp", bufs=8))
        gp = ctx.enter_context(tc.tile_pool(name="gp", bufs=12))
        hp = ctx.enter_context(tc.tile_pool(name="hp", bufs=1))
        psum = ctx.enter_context(tc.tile_pool(name="psum", bufs=8, space="PSUM"))
        dram = ctx.enter_context(tc.tile_pool(name="dram", bufs=1, space="DRAM"))
        pools = (wxp, xpp, gp, hp, psum)

        xt = const.tile([P, 3 * NXT], BF, name="xt")
        nc.sync.dma_start(xt[:], xT_d[:])
        bias = const.tile([P, 32], F32, name="bias_sb")
        nc.sync.dma_start(bias[:], bias_d[:])
        ident = const.tile([P, P], BF, name="ident_sb")
        nc.sync.dma_start(ident[:], ident_d[:])
        wh = const.tile([P, 4 * MT * MEM], BF, name="wh_sb")
        for gi in range(4):
            s = gi * MT * MEM
            nc.sync.dma_start(wh[:, s : s + MT * MEM], whT_d[:, s : s + MT * MEM])

        xproj = dram.tile([P, 32 * NXT], BF, name="xproj")

        h_ch, c_ch = _emit_xproj_and_leaf(nc, wh, xt, bias, wxT_d, xproj, xtop_d, pools)

        for lvl in range(12, 2, -1):
            n = 1 << (lvl - 3)
            off = OFF[lvl]
            par = lvl & 1

            def xp_of(gi, m, off=off, n=n, lvl=lvl):
                xp = xpp.tile([P, n], BF, tag="xp", bufs=8, name=f"xpl_{lvl}_{gi}_{m}")
                nc.sync.dma_start(xp[:], xproj[:, (gi * MT + m) * NXT + off : (gi * MT + m) * NXT + off + n])
                return xp[:]

            h_ch, c_ch = _emit_level(nc, wh, ident, h_ch, c_ch, n, xp_of, par, pools, lvl)

        out32 = gp.tile([P, 16], F32, tag="g", bufs=12, name="out32")
        for m in range(MT):
            nc.vector.tensor_copy(out32[:, m : m + 1], h_ch[m][:])
            nc.vector.tensor_copy(out32[:, 8 + m : 9 + m], c_ch[m][:])
        nc.sync.dma_start(h3c3_d[:], out32[:])
    nc.compile()
    return nc


def build_stage_b():
    nc = bacc.Bacc("TRN2", target_bir_lowering=False, debug=False, num_devices=1)
    h3_d = nc.dram_tensor("h3", [P, MT * 8], BF, kind="ExternalInput").ap()
    c3_d = nc.dram_tensor("c3", [P, MT * 8], BF, kind="ExternalInput").ap()
    whT_d = nc.dram_tensor("whT", [P, 4 * MT * MEM], BF, kind="ExternalInput").ap()
    xtop_d = nc.dram_tensor("xtop", [P, 32 * NTOP], BF, kind="ExternalInput").ap()
    ident_d = nc.dram_tensor("ident", [P, P], BF, kind="ExternalInput").ap()
    root_d = nc.dram_tensor("root", [P, 16], F32, kind="ExternalOutput").ap()

    TOFF = {2: 0, 1: 4, 0: 6}

    with tile.TileContext(nc) as tc, ExitStack() as ctx:
        const = ctx.enter_context(tc.tile_pool(name="const", bufs=1))
        wxp = ctx.enter_context(tc.tile_pool(name="wxp", bufs=6))
        xpp = ctx.enter_context(tc.tile_pool(name="xpp", bufs=8))
        gp = ctx.enter_context(tc.tile_pool(name="gp", bufs=12))
        hp = ctx.enter_context(tc.tile_pool(name="hp", bufs=1))
        psum = ctx.enter_context(tc.tile_pool(name="psum", bufs=8, space="PSUM"))
        pools = (wxp, xpp, gp, hp, psum)

        wh = const.tile([P, 4 * MT * MEM], BF, name="wh_sb")
        for gi in range(4):
            s = gi * MT * MEM
            nc.sync.dma_start(wh[:, s : s + MT * MEM], whT_d[:, s : s + MT * MEM])
        ident = const.tile([P, P], BF, name="ident_sb")
        nc.sync.dma_start(ident[:], ident_d[:])
        xtop = const.tile([P, 32 * NTOP], BF, name="xtop_sb")
        nc.sync.dma_start(xtop[:], xtop_d[:])

        h_ch, c_ch = [], []
        for m in range(MT):
            ht = hp.tile([P, 8], BF, tag=f"hin_{m}", name=f"hin_{m}")
            nc.sync.dma_start(ht[:], h3_d[:, m * 8 : (m + 1) * 8])
            ct = hp.tile([P, 8], BF, tag=f"cin_{m}", name=f"cin_{m}")
            nc.sync.dma_start(ct[:], c3_d[:, m * 8 : (m + 1) * 8])
            h_ch.append(ht)
            c_ch.append(ct)

        for lvl in range(2, -1, -1):
            n = 1 << lvl
            toff = TOFF[lvl]
            par = lvl & 1

            def xp_of(gi, m, toff=toff, n=n):
                return xtop[:, (gi * MT + m) * NTOP + toff : (gi * MT + m) * NTOP + toff + n]

            h_ch, c_ch = _emit_level(nc, wh, ident, h_ch, c_ch, n, xp_of, par, pools, lvl)

        out32 = gp.tile([P, 16], F32, tag="g", bufs=12, name="out32")
        for m in range(MT):
            nc.vector.tensor_copy(out32[:, m : m + 1], c_ch[m][:])
            nc.vector.tensor_copy(out32[:, 8 + m : 9 + m], h_ch[m][:])
        nc.sync.dma_start(root_d[:], out32[:])
    nc.compile()
    return nc


_CACHE = {}


def _get_programs():
    if "a" not in _CACHE:
        _CACHE["a"] = build_stage_a()
        _CACHE["b"] = build_stage_b()
    return _CACHE["a"], _CACHE["b"]


def _prep_host_inputs(embs, Ws, bs):
    """Build the per-core stage-A input maps (shared weight arrays)."""
    wxT = np.zeros((P, 4 * 3 * MEM), BF16)
    whT = np.zeros((P, 4 * MT * MEM), BF16)
    bias = np.zeros((P, 32), np.float32)
    for gi, g in enumerate(GATES):
        WxT = Ws[g + "x"].T.astype(BF16)  # [300, 1024]
        for kx in range(3):
            rows = WxT[kx * P : (kx + 1) * P]
            wxT[: rows.shape[0], (gi * 3 + kx) * MEM : (gi * 3 + kx + 1) * MEM] = rows
        WhT = Ws[g + "h"].T.astype(BF16)  # [1024, 1024]
        for k in range(MT):
            whT[:, (gi * MT + k) * MEM : (gi * MT + k + 1) * MEM] = WhT[k * P : (k + 1) * P]
        bias[:, gi * MT : (gi + 1) * MT] = bs[g].reshape(MT, P).T
    ident = np.eye(P, dtype=BF16)

    # top node x columns: heap order [3,4,5,6, 1,2, 0]
    x_top = embs[[3, 4, 5, 6, 1, 2, 0]].T  # [300, 7]
    in_maps = []
    for k in range(NCORE):
        cols = []
        for lvl in range(DEPTH - 1, 2, -1):
            n = 1 << lvl
            nl = n // NCORE
            cols.append(embs[n - 1 + k * nl : n - 1 + (k + 1) * nl].T)
        x_all = np.concatenate(cols + [x_top], axis=1)  # [300, 2054]
        xT = np.zeros((P, 3 * NXT), BF16)
        for kx in range(3):
            rows = x_all[kx * P : (kx + 1) * P].astype(BF16)
            xT[: rows.shape[0], kx * NXT : (kx + 1) * NXT] = rows
        in_maps.append({"xT": xT, "wxT": wxT, "whT": whT, "bias": bias, "ident": ident})
    return in_maps, whT, ident


def kernel(**inputs):
    embs = np.asarray(inputs["embs"], dtype=np.float32)
    depth = int(np.asarray(inputs["depth"]))
    assert depth == DEPTH and embs.shape == (2**DEPTH - 1, IN)
    Ws = {g + s: np.asarray(inputs["W" + g + s], dtype=np.float32) for g in GATES for s in "xh"}
    bs = {g: np.asarray(inputs["b" + g + "x"]) + np.asarray(inputs["b" + g + "h"]) for g in GATES}

    nc_a, nc_b = _get_programs()
    in_maps, whT, ident = _prep_host_inputs(embs, Ws, bs)
    res_a = run_bass_kernel_spmd(nc_a, in_maps, core_ids=list(range(NCORE))).results

    h3 = np.zeros((P, MT * 8), BF16)
    c3 = np.zeros((P, MT * 8), BF16)
    for j in range(NCORE):
        hc = res_a[j]["h3c3"]  # [128, 16] f32
        for m in range(MT):
            h3[:, m * 8 + j] = hc[:, m].astype(BF16)
            c3[:, m * 8 + j] = hc[:, 8 + m].astype(BF16)
    xtop = res_a[0]["xtop"]

    in_b = {"h3": h3, "c3": c3, "whT": whT, "xtop": np.asarray(xtop, dtype=BF16), "ident": ident}
    res_b = run_bass_kernel_spmd(nc_b, [in_b], core_ids=[0]).results
    root = res_b[0]["root"]  # [128, 16] f32: cols 0..7 = c M-tiles, 8..15 = h
    c_root = root[:, :8].T.reshape(MEM)
    h_root = root[:, 8:].T.reshape(MEM)
    return np.stack([c_root, h_root]).astype(np.float32)


# revision 5
# speedup vs baseline: 1.1068x; 1.1068x over previous
"""Child-Sum TreeLSTM (perfect binary tree, depth 14) on 8 Trainium2 NeuronCores.

Strategy
--------
Heap-order contiguous node sharding: core k owns nodes [k*n/8, (k+1)*n/8) of
every level lvl >= 3 (n = 2^lvl). Children of a core's node range at level lvl
are exactly its node range at level lvl+1, so levels 13..3 run with zero
cross-core communication (stage A, SPMD on 8 cores). Levels 2..0 (7 nodes) run
on core 0 (stage B) from the gathered 8 level-3 (h, c) states.

Within a core, state is transposed: [mem_dim(1024) partitions x n nodes], one
SBUF tile [128, 8*n] per level (M-tile m of the mem dim = col block m). Per
level, gate pre-activations accumulate in PSUM: 8 K-chunk matmuls against Wh
plus an identity-matmul that adds the precomputed x-projection (biases baked
in). All matmul inputs bf16 (fp32 PSUM accumulation); stored states bf16.
x-projections for all nodes are computed upfront in one dense pass (leaf level
fused into it): big levels staged in DRAM, small levels (9..3) kept resident
in SBUF, top-7 nodes exported for stage B.
"""

import numpy as np
import ml_dtypes
from contextlib import ExitStack

import concourse.bass as bass
import concourse.tile as tile
from concourse import bacc, mybir
from concourse.bass_utils import run_bass_kernel_spmd

BF16 = ml_dtypes.bfloat16
P = 128
MEM = 1024
IN = 300
DEPTH = 14
NCORE = 8
MT = MEM // P  # 8 M-tiles of the mem dim

NX = 2047  # per-core nodes, levels 13..3
NTOP = 7  # top nodes (levels 2..0)
NXT = NX + NTOP  # 2054
NSM = 127  # small-level nodes (levels 9..3), kept SBUF-resident
SM0 = 1920  # first small-level node col

GATES = "ifou"
SIG = mybir.ActivationFunctionType.Sigmoid
TANH = mybir.ActivationFunctionType.Tanh
IDENT = mybir.ActivationFunctionType.Identity
ACT_FN = {"i": SIG, "f": SIG, "o": SIG, "u": TANH}

# per-core column offset of level lvl within the node axis (levels 13..3)
OFF = {13: 0}
for _lvl in range(12, 2, -1):
    OFF[_lvl] = OFF[_lvl + 1] + (2 ** (_lvl + 1)) // NCORE

LEAF_CHUNKS = [(0, 512), (512, 512)]  # lvl 13, fused, i/o/u only
MID_CHUNKS = [(1024, 512), (1536, 384)]  # lvl 12..10 -> DRAM
SMALL_CHUNK = (SM0, NSM)  # lvl 9..3 -> resident SBUF
TOP_CHUNK = (NX, NTOP)  # lvl 2..0 -> xtop output

F32 = mybir.dt.float32
BF = mybir.dt.bfloat16

COMBINED_IOU = False  # pack i/o/u pre-acts in one PSUM tile (small levels)
FUSED_F = True  # single interleaved [L R L R] f-gate PSUM (levels with 2n<=512)


def _emit_xproj_and_leaf(nc, xt, bias, wxT_d, xproj, xres, xtop_sb, pools):
    """x-projection pass (bias baked in) with leaf level (13) fused.
    Writes mid chunks to DRAM `xproj`, small chunk to resident SBUF `xres`
    ((g,m)-major: col (gi*8+m)*NSM + ...), top chunk to `xtop_sb`.
    Returns leaf state tiles h13, c13 [128, 8*1024] (bf16)."""
    xpp, gp, hp, psum = pools
    h13 = hp.tile([P, MT * 1024], BF, tag="h_odd", name="h13")
    c13 = hp.tile([P, MT * 1024], BF, tag="c_odd", name="c13")
    wx3 = wxT_d.rearrange("p (b c) -> p b c", c=MEM)
    for m in range(MT):
        leaf = {}
        for g in "iouf":
            gi = GATES.index(g)
            wx = gp.tile([P, 3 * P], BF, tag="wx", bufs=6, name=f"wx_{m}_{g}")
            nc.sync.dma_start(
                wx[:].rearrange("p (b c) -> p b c", c=P),
                wx3[:, gi * 3 : gi * 3 + 3, m * P : (m + 1) * P],
            )
            chunks = (LEAF_CHUNKS if g != "f" else []) + MID_CHUNKS + [SMALL_CHUNK, TOP_CHUNK]
            for c0, ncol in chunks:
                ps = psum.tile([P, ncol], F32, tag="ps", bufs=8, name=f"ps_{m}_{g}{c0}")
                for kx in range(3):
                    nc.tensor.matmul(
                        ps[:],
                        wx[:, kx * P : (kx + 1) * P],
                        xt[:, kx * NXT + c0 : kx * NXT + c0 + ncol],
                        start=(kx == 0),
                        stop=(kx == 2),
                    )
                bias_ap = bias[:, gi * MT + m : gi * MT + m + 1]
                if (c0, ncol) in LEAF_CHUNKS:
                    tg = gp.tile([P, ncol], F32, tag="g", bufs=12, name=f"lf_{m}_{g}{c0}")
                    nc.scalar.activation(tg[:], ps[:], ACT_FN[g], bias=bias_ap)
                    leaf[(g, c0)] = tg
                elif (c0, ncol) == SMALL_CHUNK:
                    nc.scalar.activation(
                        xres[:, (gi * MT + m) * NSM : (gi * MT + m + 1) * NSM], ps[:], IDENT, bias=bias_ap
                    )
                elif (c0, ncol) == TOP_CHUNK:
                    nc.scalar.activation(
                        xtop_sb[:, (gi * MT + m) * NTOP : (gi * MT + m + 1) * NTOP], ps[:], IDENT, bias=bias_ap
                    )
                else:
                    xp = xpp.tile([P, ncol], BF, tag="xp", bufs=8, name=f"xp_{m}_{g}{c0}")
                    nc.scalar.activation(xp[:], ps[:], IDENT, bias=bias_ap)
                    nc.sync.dma_start(
                        xproj[:, (gi * MT + m) * 896 + (c0 - 1024) : (gi * MT + m) * 896 + (c0 - 1024) + ncol],
                        xp[:],
                    )
        # leaf combine: c = i*u ; h = o*tanh(c)
        for c0, ncol in LEAF_CHUNKS:
            cs = c13[:, m * 1024 + c0 : m * 1024 + c0 + ncol]
            nc.vector.tensor_mul(cs, leaf[("i", c0)][:], leaf[("u", c0)][:])
            th = gp.tile([P, ncol], F32, tag="g", bufs=12, name=f"th13_{m}_{c0}")
            nc.scalar.activation(th[:], cs, TANH)
            nc.vector.tensor_mul(h13[:, m * 1024 + c0 : m * 1024 + c0 + ncol], leaf[("o", c0)][:], th[:])
    return h13, c13


def _emit_level(nc, wh, ident, h_ch, c_ch, n, xp_of, par, pools, lvl):
    """One non-leaf level. h_ch/c_ch: single tiles [128, 8*2n] bf16.
    xp_of(gi, m) -> SBUF AP [128, n] with xproj+bias for that gate/M-tile.
    Returns (h_new, c_new) single tiles [128, 8*n] bf16."""
    xpp, gp, hp, psum = pools
    n2 = 2 * n
    combined_iou = COMBINED_IOU and 3 * n <= 512
    fused_f = FUSED_F and n2 <= 512

    def wh_ap(gi, k, m):
        return wh[:, (gi * MT + k) * MEM + m * P : (gi * MT + k) * MEM + (m + 1) * P]

    hs = hp.tile([P, MT * n], BF, tag=f"hs_{par}", name=f"hs_{lvl}")
    nc.vector.tensor_add(hs[:], h_ch[:, 0 : MT * n2 : 2], h_ch[:, 1 : MT * n2 : 2])
    h_new = hp.tile([P, MT * n], BF, tag=f"h_{'odd' if par else 'even'}", name=f"h_{lvl}")
    c_new = hp.tile([P, MT * n], BF, tag=f"c_{'odd' if par else 'even'}", name=f"c_{lvl}")

    for m in range(MT):
        gio = {}
        if combined_iou:
            ps3 = psum.tile([P, 3 * n], F32, tag="ps", bufs=8, name=f"ps3_{lvl}_{m}")
            for k in range(MT):
                hsk = hs[:, k * n : (k + 1) * n]
                for sl, g in enumerate("iou"):
                    nc.tensor.matmul(
                        ps3[:, sl * n : (sl + 1) * n], wh_ap(GATES.index(g), k, m), hsk, start=(k == 0), stop=False
                    )
            for sl, g in enumerate("iou"):
                nc.tensor.matmul(
                    ps3[:, sl * n : (sl + 1) * n], ident[:], xp_of(GATES.index(g), m), start=False, stop=True
                )
            gt = gp.tile([P, 3 * n], F32, tag="g", bufs=12, name=f"gt_{lvl}_{m}")
            nc.scalar.activation(gt[:, 0 : 2 * n], ps3[:, 0 : 2 * n], SIG)
            nc.scalar.activation(gt[:, 2 * n : 3 * n], ps3[:, 2 * n : 3 * n], TANH)
            gio["i"], gio["o"], gio["u"] = gt[:, 0:n], gt[:, n : 2 * n], gt[:, 2 * n : 3 * n]
        else:
            for g in "iou":
                gi = GATES.index(g)
                ps = psum.tile([P, n], F32, tag="ps", bufs=8, name=f"ps_{lvl}_{m}{g}")
                for k in range(MT):
                    nc.tensor.matmul(ps[:], wh_ap(gi, k, m), hs[:, k * n : (k + 1) * n], start=(k == 0), stop=False)
                nc.tensor.matmul(ps[:], ident[:], xp_of(gi, m), start=False, stop=True)
                tg = gp.tile([P, n], F32, tag="g", bufs=12, name=f"t{g}_{lvl}_{m}")
                nc.scalar.activation(tg[:], ps[:], ACT_FN[g])
                gio[g] = tg[:]

        gi = GATES.index("f")
        if fused_f:
            # interleaved [L R L R ...] pre-acts, contiguous children rhs
            psf = psum.tile([P, n2], F32, tag="ps", bufs=8, name=f"psf_{lvl}_{m}")
            for k in range(MT):
                nc.tensor.matmul(psf[:], wh_ap(gi, k, m), h_ch[:, k * n2 : (k + 1) * n2], start=(k == 0), stop=False)
            xpf = xp_of(gi, m)
            xpf2 = xpp.tile([P, n2], BF, tag="xpf2", bufs=4, name=f"xpf2_{lvl}_{m}")
            nc.vector.tensor_copy(xpf2[:, 0:n2:2], xpf)
            nc.vector.tensor_copy(xpf2[:, 1:n2:2], xpf)
            nc.tensor.matmul(psf[:], ident[:], xpf2[:], start=False, stop=True)
            ff = gp.tile([P, n2], F32, tag="g", bufs=12, name=f"ff_{lvl}_{m}")
            nc.scalar.activation(ff[:], psf[:], SIG)
            prod = gp.tile([P, n2], F32, tag="g", bufs=12, name=f"prod_{lvl}_{m}")
            nc.vector.tensor_mul(prod[:], ff[:], c_ch[:, m * n2 : (m + 1) * n2])
            fc = gp.tile([P, n], F32, tag="g", bufs=12, name=f"fc_{lvl}_{m}")
            nc.vector.tensor_add(fc[:], prod[:, 0:n2:2], prod[:, 1:n2:2])
        else:
            psL = psum.tile([P, n], F32, tag="ps", bufs=8, name=f"psL_{lvl}_{m}")
            psR = psum.tile([P, n], F32, tag="ps", bufs=8, name=f"psR_{lvl}_{m}")
            for k in range(MT):
                w = wh_ap(gi, k, m)
                nc.tensor.matmul(psL[:], w, h_ch[:, k * n2 : (k + 1) * n2 : 2], start=(k == 0), stop=False)
                nc.tensor.matmul(psR[:], w, h_ch[:, k * n2 + 1 : (k + 1) * n2 : 2], start=(k == 0), stop=False)
            xpf = xp_of(gi, m)
            nc.tensor.matmul(psL[:], ident[:], xpf, start=False, stop=True)
            nc.tensor.matmul(psR[:], ident[:], xpf, start=False, stop=True)
            fL = gp.tile([P, n], F32, tag="g", bufs=12, name=f"fL_{lvl}_{m}")
            nc.scalar.activation(fL[:], psL[:], SIG)
            fR = gp.tile([P, n], F32, tag="g", bufs=12, name=f"fR_{lvl}_{m}")
            nc.scalar.activation(fR[:], psR[:], SIG)
            t1 = gp.tile([P, n], F32, tag="g", bufs=12, name=f"t1_{lvl}_{m}")
            nc.vector.tensor_mul(t1[:], fL[:], c_ch[:, m * n2 : (m + 1) * n2 : 2])
            t2 = gp.tile([P, n], F32, tag="g", bufs=12, name=f"t2_{lvl}_{m}")
            nc.vector.tensor_mul(t2[:], fR[:], c_ch[:, m * n2 + 1 : (m + 1) * n2 : 2])
            fc = gp.tile([P, n], F32, tag="g", bufs=12, name=f"fc_{lvl}_{m}")
            nc.vector.tensor_add(fc[:], t1[:], t2[:])

        tiu = gp.tile([P, n], F32, tag="g", bufs=12, name=f"tiu_{lvl}_{m}")
        nc.vector.tensor_mul(tiu[:], gio["i"], gio["u"])
        cm = c_new[:, m * n : (m + 1) * n]
        nc.vector.tensor_add(cm, tiu[:], fc[:])
        th = gp.tile([P, n], F32, tag="g", bufs=12, name=f"th_{lvl}_{m}")
        nc.scalar.activation(th[:], cm, TANH)
        nc.vector.tensor_mul(h_new[:, m * n : (m + 1) * n], gio["o"], th[:])
    return h_new, c_new


def build_stage_a():
    nc = bacc.Bacc("TRN2", target_bir_lowering=False, debug=False, num_devices=NCORE)
    xT_d = nc.dram_tensor("xT", [P, 3 * NXT], BF, kind="ExternalInput").ap()
    wxT_d = nc.dram_tensor("wxT", [P, 4 * 3 * MEM], BF, kind="ExternalInput").ap()
    whT_d = nc.dram_tensor("whT", [P, 4 * MT * MEM], BF, kind="ExternalInput").ap()
    bias_d = nc.dram_tensor("bias", [P, 32], F32, kind="ExternalInput").ap()
    ident_d = nc.dram_tensor("ident", [P, P], BF, kind="ExternalInput").ap()
    h3c3_d = nc.dram_tensor("h3c3", [P, 16], F32, kind="ExternalOutput").ap()
    xtop_d = nc.dram_tensor("xtop", [P, 32 * NTOP], BF, kind="ExternalOutput").ap()

    with tile.TileContext(nc) as tc, ExitStack() as ctx:
        const = ctx.enter_context(tc.tile_pool(name="const", bufs=1))
        xpp = ctx.enter_context(tc.tile_pool(name="xpp", bufs=8))
        gp = ctx.enter_context(tc.tile_pool(name="gp", bufs=12))
        hp = ctx.enter_context(tc.tile_pool(name="hp", bufs=1))
        psum = ctx.enter_context(tc.tile_pool(name="psum", bufs=8, space="PSUM"))
        dram = ctx.enter_context(tc.tile_pool(name="dram", bufs=1, space="DRAM"))
        pools = (xpp, gp, hp, psum)

        xt = const.tile([P, 3 * NXT], BF, name="xt")
        nc.sync.dma_start(xt[:], xT_d[:])
        bias = const.tile([P, 32], F32, name="bias_sb")
        nc.sync.dma_start(bias[:], bias_d[:])
        ident = const.tile([P, P], BF, name="ident_sb")
        nc.sync.dma_start(ident[:], ident_d[:])
        xres = const.tile([P, 32 * NSM], BF, name="xres")
        xtop_sb = const.tile([P, 32 * NTOP], BF, name="xtop_sb")
        xproj = dram.tile([P, 32 * 896], BF, name="xproj")  # mid levels 12..10

        h_ch, c_ch = _emit_xproj_and_leaf(nc, xt, bias, wxT_d, xproj, xres, xtop_sb, pools)
        nc.sync.dma_start(xtop_d[:], xtop_sb[:])

        # weights for the h-GEMMs load during the xproj/leaf pass
        wh = const.tile([P, 4 * MT * MEM], BF, name="wh_sb")
        for gi in range(4):
            s = gi * MT * MEM
            nc.sync.dma_start(wh[:, s : s + MT * MEM], whT_d[:, s : s + MT * MEM])

        for lvl in range(12, 2, -1):
            n = 1 << (lvl - 3)
            par = lvl & 1
            if lvl >= 10:
                off = OFF[lvl] - 1024

                def xp_of(gi, m, off=off, n=n, lvl=lvl):
                    xp = xpp.tile([P, n], BF, tag="xp", bufs=8, name=f"xpl_{lvl}_{gi}_{m}")
                    nc.sync.dma_start(
                        xp[:], xproj[:, (gi * MT + m) * 896 + off : (gi * MT + m) * 896 + off + n]
                    )
                    return xp[:]

            else:
                off = OFF[lvl] - SM0

                def xp_of(gi, m, off=off, n=n):
                    return xres[:, (gi * MT + m) * NSM + off : (gi * MT + m) * NSM + off + n]

            h_ch, c_ch = _emit_level(nc, wh, ident, h_ch, c_ch, n, xp_of, par, pools, lvl)

        out32 = gp.tile([P, 16], F32, tag="g", bufs=12, name="out32")
        nc.vector.tensor_copy(out32[:, 0:8], h_ch[:])
        nc.vector.tensor_copy(out32[:, 8:16], c_ch[:])
        nc.sync.dma_start(h3c3_d[:], out32[:])
    nc.compile()
    return nc


def build_stage_b():
    nc = bacc.Bacc("TRN2", target_bir_lowering=False, debug=False, num_devices=1)
    h3_d = nc.dram_tensor("h3", [P, MT * 8], BF, kind="ExternalInput").ap()
    c3_d = nc.dram_tensor("c3", [P, MT * 8], BF, kind="ExternalInput").ap()
    whT_d = nc.dram_tensor("whT", [P, 4 * MT * MEM], BF, kind="ExternalInput").ap()
    xtop_d = nc.dram_tensor("xtop", [P, 32 * NTOP], BF, kind="ExternalInput").ap()
    ident_d = nc.dram_tensor("ident", [P, P], BF, kind="ExternalInput").ap()
    root_d = nc.dram_tensor("root", [P, 16], F32, kind="ExternalOutput").ap()

    TOFF = {2: 0, 1: 4, 0: 6}

    with tile.TileContext(nc) as tc, ExitStack() as ctx:
        const = ctx.enter_context(tc.tile_pool(name="const", bufs=1))
        xpp = ctx.enter_context(tc.tile_pool(name="xpp", bufs=8))
        gp = ctx.enter_context(tc.tile_pool(name="gp", bufs=12))
        hp = ctx.enter_context(tc.tile_pool(name="hp", bufs=1))
        psum = ctx.enter_context(tc.tile_pool(name="psum", bufs=8, space="PSUM"))
        pools = (xpp, gp, hp, psum)

        wh = const.tile([P, 4 * MT * MEM], BF, name="wh_sb")
        for gi in range(4):
            s = gi * MT * MEM
            nc.sync.dma_start(wh[:, s : s + MT * MEM], whT_d[:, s : s + MT * MEM])
        ident = const.tile([P, P], BF, name="ident_sb")
        nc.sync.dma_start(ident[:], ident_d[:])
        xtop = const.tile([P, 32 * NTOP], BF, name="xtop_sb")
        nc.sync.dma_start(xtop[:], xtop_d[:])

        h_ch = hp.tile([P, MT * 8], BF, tag="h_odd", name="h_in")
        nc.sync.dma_start(h_ch[:], h3_d[:])
        c_ch = hp.tile([P, MT * 8], BF, tag="c_odd", name="c_in")
        nc.sync.dma_start(c_ch[:], c3_d[:])

        for lvl in range(2, -1, -1):
            n = 1 << lvl
            toff = TOFF[lvl]
            par = lvl & 1

            def xp_of(gi, m, toff=toff, n=n):
                return xtop[:, (gi * MT + m) * NTOP + toff : (gi * MT + m) * NTOP + toff + n]

            h_ch, c_ch = _emit_level(nc, wh, ident, h_ch, c_ch, n, xp_of, par, pools, lvl)

        out32 = gp.tile([P, 16], F32, tag="g", bufs=12, name="out32")
        nc.vector.tensor_copy(out32[:, 0:8], c_ch[:])
        nc.vector.tensor_copy(out32[:, 8:16], h_ch[:])
        nc.sync.dma_start(root_d[:], out32[:])
    nc.compile()
    return nc


_CACHE = {}


def _get_programs():
    if "a" not in _CACHE:
        _CACHE["a"] = build_stage_a()
        _CACHE["b"] = build_stage_b()
    return _CACHE["a"], _CACHE["b"]


def _prep_host_inputs(embs, Ws, bs):
    wxT = np.zeros((P, 4 * 3 * MEM), BF16)
    whT = np.zeros((P, 4 * MT * MEM), BF16)
    bias = np.zeros((P, 32), np.float32)
    for gi, g in enumerate(GATES):
        WxT = Ws[g + "x"].T.astype(BF16)  # [300, 1024]
        for kx in range(3):
            rows = WxT[kx * P : (kx + 1) * P]
            wxT[: rows.shape[0], (gi * 3 + kx) * MEM : (gi * 3 + kx + 1) * MEM] = rows
        WhT = Ws[g + "h"].T.astype(BF16)  # [1024, 1024]
        for k in range(MT):
            whT[:, (gi * MT + k) * MEM : (gi * MT + k + 1) * MEM] = WhT[k * P : (k + 1) * P]
        bias[:, gi * MT : (gi + 1) * MT] = bs[g].reshape(MT, P).T
    ident = np.eye(P, dtype=BF16)

    x_top = embs[[3, 4, 5, 6, 1, 2, 0]].T  # [300, 7], heap order per level
    in_maps = []
    for k in range(NCORE):
        cols = []
        for lvl in range(DEPTH - 1, 2, -1):
            n = 1 << lvl
            nl = n // NCORE
            cols.append(embs[n - 1 + k * nl : n - 1 + (k + 1) * nl].T)
        x_all = np.concatenate(cols + [x_top], axis=1)  # [300, 2054]
        xT = np.zeros((P, 3 * NXT), BF16)
        for kx in range(3):
            rows = x_all[kx * P : (kx + 1) * P].astype(BF16)
            xT[: rows.shape[0], kx * NXT : (kx + 1) * NXT] = rows
        in_maps.append({"xT": xT, "wxT": wxT, "whT": whT, "bias": bias, "ident": ident})
    return in_maps, whT, ident


def kernel(**inputs):
    embs = np.asarray(inputs["embs"], dtype=np.float32)
    depth = int(np.asarray(inputs["depth"]))
    assert depth == DEPTH and embs.shape == (2**DEPTH - 1, IN)
    Ws = {g + s: np.asarray(inputs["W" + g + s], dtype=np.float32) for g in GATES for s in "xh"}
    bs = {g: np.asarray(inputs["b" + g + "x"]) + np.asarray(inputs["b" + g + "h"]) for g in GATES}

    nc_a, nc_b = _get_programs()
    in_maps, whT, ident = _prep_host_inputs(embs, Ws, bs)
    res_a = run_bass_kernel_spmd(nc_a, in_maps, core_ids=list(range(NCORE))).results

    h3 = np.zeros((P, MT * 8), BF16)
    c3 = np.zeros((P, MT * 8), BF16)
    for j in range(NCORE):
        hc = res_a[j]["h3c3"]  # [128, 16] f32
        for m in range(MT):
            h3[:, m * 8 + j] = hc[:, m].astype(BF16)
            c3[:, m * 8 + j] = hc[:, 8 + m].astype(BF16)
    xtop = res_a[0]["xtop"]

    in_b = {"h3": h3, "c3": c3, "whT": whT, "xtop": np.asarray(xtop, dtype=BF16), "ident": ident}
    res_b = run_bass_kernel_spmd(nc_b, [in_b], core_ids=[0]).results
    root = res_b[0]["root"]  # [128, 16] f32: cols 0..7 = c M-tiles, 8..15 = h
    c_root = root[:, :8].T.reshape(MEM)
    h_root = root[:, 8:].T.reshape(MEM)
    return np.stack([c_root, h_root]).astype(np.float32)


# revision 9
# speedup vs baseline: 1.3522x; 1.2217x over previous
"""Child-Sum TreeLSTM (perfect binary tree, depth 14) on 8 Trainium2 NeuronCores.

Strategy
--------
Heap-order contiguous node sharding: core k owns nodes [k*n/8, (k+1)*n/8) of
every level lvl >= 3 (n = 2^lvl). Children of a core's node range at level lvl
are exactly its node range at level lvl+1, so levels 13..3 run with zero
cross-core communication (stage A, SPMD on 8 cores). Levels 2..0 (7 nodes) run
on core 0 (stage B) from the gathered 8 level-3 (h, c) states.

Within a core, state is transposed: [mem_dim(1024) partitions x n nodes], one
SBUF tile [128, 8*n] per level (M-tile m of the mem dim = col block m). Per
level, gate pre-activations accumulate in PSUM: 8 K-chunk matmuls against Wh
plus an identity-matmul that adds the precomputed x-projection (biases baked
in). All matmul inputs bf16 (fp32 PSUM accumulation); stored states bf16.
x-projections for all nodes are computed upfront in one dense pass (leaf level
fused into it): big levels staged in DRAM, small levels (9..3) kept resident
in SBUF, top-7 nodes exported for stage B.
"""

import numpy as np
import ml_dtypes
from contextlib import ExitStack

import concourse.bass as bass
import concourse.tile as tile
from concourse import bacc, mybir
from concourse.bass_utils import run_bass_kernel_spmd

BF16 = ml_dtypes.bfloat16
P = 128
MEM = 1024
IN = 300
DEPTH = 14
NCORE = 8
MT = MEM // P  # 8 M-tiles of the mem dim

NX = 2047  # per-core nodes, levels 13..3
NTOP = 7  # top nodes (levels 2..0)
NXT = NX + NTOP  # 2054
NSM = 127  # small-level nodes (levels 9..3), kept SBUF-resident
SM0 = 1920  # first small-level node col

GATES = "ifou"
SIG = mybir.ActivationFunctionType.Sigmoid
TANH = mybir.ActivationFunctionType.Tanh
IDENT = mybir.ActivationFunctionType.Identity
ACT_FN = {"i": SIG, "f": SIG, "o": SIG, "u": TANH}

# per-core column offset of level lvl within the node axis (levels 13..3)
OFF = {13: 0}
for _lvl in range(12, 2, -1):
    OFF[_lvl] = OFF[_lvl + 1] + (2 ** (_lvl + 1)) // NCORE

LEAF_CHUNKS = [(0, 512), (512, 512)]  # lvl 13, fused, i/o/u only
MID_CHUNKS = [(1024, 512), (1536, 384)]  # lvl 12..10 -> DRAM
SMALL_CHUNK = (SM0, NSM)  # lvl 9..3 -> resident SBUF
TOP_CHUNK = (NX, NTOP)  # lvl 2..0 -> xtop output

F32 = mybir.dt.float32
BF = mybir.dt.bfloat16

COMBINED_IOU = False  # pack i/o/u pre-acts in one PSUM tile (small levels)
FUSED_F = True  # single interleaved [L R L R] f-gate PSUM (levels with 2n<=512)


def _emit_xproj_and_leaf(nc, xt, bias, wxT_d, xproj, xres, xtop_sb, pools):
    """x-projection pass (bias baked in) with leaf level (13) fused.
    Writes mid chunks to DRAM `xproj`, small chunk to resident SBUF `xres`
    ((g,m)-major: col (gi*8+m)*NSM + ...), top chunk to `xtop_sb`.
    Returns leaf state tiles h13, c13 [128, 8*1024] (bf16)."""
    xpp, gp, hp, psum = pools
    h13 = hp.tile([P, MT * 1024], BF, tag="h_odd", name="h13")
    c13 = hp.tile([P, MT * 1024], BF, tag="c_odd", name="c13")
    wx3 = wxT_d.rearrange("p (b c) -> p b c", c=MEM)
    for m in range(MT):
        leaf = {}
        for g in "iouf":
            gi = GATES.index(g)
            wx = gp.tile([P, 3 * P], BF, tag="wx", bufs=6, name=f"wx_{m}_{g}")
            nc.sync.dma_start(
                wx[:].rearrange("p (b c) -> p b c", c=P),
                wx3[:, gi * 3 : gi * 3 + 3, m * P : (m + 1) * P],
            )
            chunks = (LEAF_CHUNKS if g != "f" else []) + MID_CHUNKS + [SMALL_CHUNK, TOP_CHUNK]
            for c0, ncol in chunks:
                ps = psum.tile([P, ncol], F32, tag="ps", bufs=8, name=f"ps_{m}_{g}{c0}")
                for kx in range(3):
                    nc.tensor.matmul(
                        ps[:],
                        wx[:, kx * P : (kx + 1) * P],
                        xt[:, kx * NXT + c0 : kx * NXT + c0 + ncol],
                        start=(kx == 0),
                        stop=(kx == 2),
                    )
                bias_ap = bias[:, gi * MT + m : gi * MT + m + 1]
                if (c0, ncol) in LEAF_CHUNKS:
                    tg = gp.tile([P, ncol], F32, tag="g", bufs=12, name=f"lf_{m}_{g}{c0}")
                    nc.scalar.activation(tg[:], ps[:], ACT_FN[g], bias=bias_ap)
                    leaf[(g, c0)] = tg
                elif (c0, ncol) == SMALL_CHUNK:
                    nc.scalar.activation(
                        xres[:, (gi * MT + m) * NSM : (gi * MT + m + 1) * NSM], ps[:], IDENT, bias=bias_ap
                    )
                elif (c0, ncol) == TOP_CHUNK:
                    nc.scalar.activation(
                        xtop_sb[:, (gi * MT + m) * NTOP : (gi * MT + m + 1) * NTOP], ps[:], IDENT, bias=bias_ap
                    )
                else:
                    xp = xpp.tile([P, ncol], BF, tag="xp", bufs=8, name=f"xp_{m}_{g}{c0}")
                    nc.scalar.activation(xp[:], ps[:], IDENT, bias=bias_ap)
                    nc.sync.dma_start(
                        xproj[:, (gi * MT + m) * 896 + (c0 - 1024) : (gi * MT + m) * 896 + (c0 - 1024) + ncol],
                        xp[:],
                    )
        # leaf combine: c = i*u ; h = o*tanh(c)
        for c0, ncol in LEAF_CHUNKS:
            cs = c13[:, m * 1024 + c0 : m * 1024 + c0 + ncol]
            nc.vector.tensor_mul(cs, leaf[("i", c0)][:], leaf[("u", c0)][:])
            th = gp.tile([P, ncol], F32, tag="g", bufs=12, name=f"th13_{m}_{c0}")
            nc.scalar.activation(th[:], cs, TANH)
            nc.vector.tensor_mul(h13[:, m * 1024 + c0 : m * 1024 + c0 + ncol], leaf[("o", c0)][:], th[:])
    return h13, c13


def _emit_level(nc, wh, ident, h_ch, c_ch, n, xp_of, par, pools, lvl):
    """One non-leaf level. h_ch/c_ch: single tiles [128, 8*2n] bf16.
    xp_of(gi, m) -> SBUF AP [128, n] with xproj+bias for that gate/M-tile.
    Returns (h_new, c_new) single tiles [128, 8*n] bf16."""
    xpp, gp, hp, psum = pools
    n2 = 2 * n
    combined_iou = COMBINED_IOU and 3 * n <= 512
    fused_f = FUSED_F and n2 <= 512

    def wh_ap(gi, k, m):
        return wh[:, (gi * MT + k) * MEM + m * P : (gi * MT + k) * MEM + (m + 1) * P]

    h_new = hp.tile([P, MT * n], BF, tag=f"h_{'odd' if par else 'even'}", name=f"h_{lvl}")
    c_new = hp.tile([P, MT * n], BF, tag=f"c_{'odd' if par else 'even'}", name=f"c_{lvl}")

    # f-gate matmuls first: they depend only on child h, so the PE has dense
    # work at level start while the DVE computes hsum.
    ff_of, fc_done = {}, {}
    if fused_f:
        for m in range(MT):
            xpf = xp_of(GATES.index("f"), m)
            xpf2 = xpp.tile([P, n2], BF, tag="xpf2", bufs=4, name=f"xpf2_{lvl}_{m}")
            nc.vector.tensor_copy(xpf2[:, 0:n2:2], xpf)
            nc.vector.tensor_copy(xpf2[:, 1:n2:2], xpf)
            psf = psum.tile([P, n2], F32, tag="ps", bufs=8, name=f"psf_{lvl}_{m}")
            for k in range(MT):
                nc.tensor.matmul(
                    psf[:], wh_ap(GATES.index("f"), k, m), h_ch[:, k * n2 : (k + 1) * n2], start=(k == 0), stop=False
                )
            nc.tensor.matmul(psf[:], ident[:], xpf2[:], start=False, stop=True)
            ff = gp.tile([P, n2], F32, tag="g", bufs=12, name=f"ff_{lvl}_{m}")
            nc.scalar.activation(ff[:], psf[:], SIG)
            ff_of[m] = ff

    hs = hp.tile([P, MT * n], BF, tag=f"hs_{par}", name=f"hs_{lvl}")
    for k in range(MT):
        nc.vector.tensor_add(
            hs[:, k * n : (k + 1) * n], h_ch[:, k * n2 : (k + 1) * n2 : 2], h_ch[:, k * n2 + 1 : (k + 1) * n2 : 2]
        )

    for m in range(MT):
        gio = {}
        if combined_iou:
            ps3 = psum.tile([P, 3 * n], F32, tag="ps", bufs=8, name=f"ps3_{lvl}_{m}")
            for k in range(MT):
                hsk = hs[:, k * n : (k + 1) * n]
                for sl, g in enumerate("iou"):
                    nc.tensor.matmul(
                        ps3[:, sl * n : (sl + 1) * n], wh_ap(GATES.index(g), k, m), hsk, start=(k == 0), stop=False
                    )
            for sl, g in enumerate("iou"):
                nc.tensor.matmul(
                    ps3[:, sl * n : (sl + 1) * n], ident[:], xp_of(GATES.index(g), m), start=False, stop=True
                )
            gt = gp.tile([P, 3 * n], F32, tag="g", bufs=12, name=f"gt_{lvl}_{m}")
            nc.scalar.activation(gt[:, 0 : 2 * n], ps3[:, 0 : 2 * n], SIG)
            nc.scalar.activation(gt[:, 2 * n : 3 * n], ps3[:, 2 * n : 3 * n], TANH)
            gio["i"], gio["o"], gio["u"] = gt[:, 0:n], gt[:, n : 2 * n], gt[:, 2 * n : 3 * n]
        else:
            for g in "iou":
                gi = GATES.index(g)
                ps = psum.tile([P, n], F32, tag="ps", bufs=8, name=f"ps_{lvl}_{m}{g}")
                for k in range(MT):
                    nc.tensor.matmul(ps[:], wh_ap(gi, k, m), hs[:, k * n : (k + 1) * n], start=(k == 0), stop=False)
                nc.tensor.matmul(ps[:], ident[:], xp_of(gi, m), start=False, stop=True)
                tg = gp.tile([P, n], F32, tag="g", bufs=12, name=f"t{g}_{lvl}_{m}")
                nc.scalar.activation(tg[:], ps[:], ACT_FN[g])
                gio[g] = tg[:]

        gi = GATES.index("f")
        if fused_f:
            prod = gp.tile([P, n2], F32, tag="g", bufs=12, name=f"prod_{lvl}_{m}")
            nc.vector.tensor_mul(prod[:], ff_of[m][:], c_ch[:, m * n2 : (m + 1) * n2])
            fc = gp.tile([P, n], F32, tag="g", bufs=12, name=f"fc_{lvl}_{m}")
            nc.vector.tensor_add(fc[:], prod[:, 0:n2:2], prod[:, 1:n2:2])
        else:
            psL = psum.tile([P, n], F32, tag="ps", bufs=8, name=f"psL_{lvl}_{m}")
            psR = psum.tile([P, n], F32, tag="ps", bufs=8, name=f"psR_{lvl}_{m}")
            for k in range(MT):
                w = wh_ap(gi, k, m)
                nc.tensor.matmul(psL[:], w, h_ch[:, k * n2 : (k + 1) * n2 : 2], start=(k == 0), stop=False)
                nc.tensor.matmul(psR[:], w, h_ch[:, k * n2 + 1 : (k + 1) * n2 : 2], start=(k == 0), stop=False)
            xpf = xp_of(gi, m)
            nc.tensor.matmul(psL[:], ident[:], xpf, start=False, stop=True)
            nc.tensor.matmul(psR[:], ident[:], xpf, start=False, stop=True)
            fL = gp.tile([P, n], F32, tag="g", bufs=12, name=f"fL_{lvl}_{m}")
            nc.scalar.activation(fL[:], psL[:], SIG)
            fR = gp.tile([P, n], F32, tag="g", bufs=12, name=f"fR_{lvl}_{m}")
            nc.scalar.activation(fR[:], psR[:], SIG)
            t1 = gp.tile([P, n], F32, tag="g", bufs=12, name=f"t1_{lvl}_{m}")
            nc.vector.tensor_mul(t1[:], fL[:], c_ch[:, m * n2 : (m + 1) * n2 : 2])
            t2 = gp.tile([P, n], F32, tag="g", bufs=12, name=f"t2_{lvl}_{m}")
            nc.vector.tensor_mul(t2[:], fR[:], c_ch[:, m * n2 + 1 : (m + 1) * n2 : 2])
            fc = gp.tile([P, n], F32, tag="g", bufs=12, name=f"fc_{lvl}_{m}")
            nc.vector.tensor_add(fc[:], t1[:], t2[:])

        tiu = gp.tile([P, n], F32, tag="g", bufs=12, name=f"tiu_{lvl}_{m}")
        nc.vector.tensor_mul(tiu[:], gio["i"], gio["u"])
        cm = c_new[:, m * n : (m + 1) * n]
        nc.vector.tensor_add(cm, tiu[:], fc[:])
        th = gp.tile([P, n], F32, tag="g", bufs=12, name=f"th_{lvl}_{m}")
        nc.scalar.activation(th[:], cm, TANH)
        nc.vector.tensor_mul(h_new[:, m * n : (m + 1) * n], gio["o"], th[:])
    return h_new, c_new


def build_stage_a():
    nc = bacc.Bacc("TRN2", target_bir_lowering=False, debug=False, num_devices=NCORE)
    xT_d = nc.dram_tensor("xT", [P, 3 * NXT], BF, kind="ExternalInput").ap()
    wxT_d = nc.dram_tensor("wxT", [P, 4 * 3 * MEM], BF, kind="ExternalInput").ap()
    whT_d = nc.dram_tensor("whT", [P, 4 * MT * MEM], BF, kind="ExternalInput").ap()
    bias_d = nc.dram_tensor("bias", [P, 32], F32, kind="ExternalInput").ap()
    ident_d = nc.dram_tensor("ident", [P, P], BF, kind="ExternalInput").ap()
    h3c3_d = nc.dram_tensor("h3c3", [P, 16], F32, kind="ExternalOutput").ap()
    xtop_d = nc.dram_tensor("xtop", [P, 32 * NTOP], BF, kind="ExternalOutput").ap()

    with tile.TileContext(nc) as tc, ExitStack() as ctx:
        const = ctx.enter_context(tc.tile_pool(name="const", bufs=1))
        xpp = ctx.enter_context(tc.tile_pool(name="xpp", bufs=8))
        gp = ctx.enter_context(tc.tile_pool(name="gp", bufs=12))
        hp = ctx.enter_context(tc.tile_pool(name="hp", bufs=1))
        psum = ctx.enter_context(tc.tile_pool(name="psum", bufs=8, space="PSUM"))
        dram = ctx.enter_context(tc.tile_pool(name="dram", bufs=1, space="DRAM"))
        pools = (xpp, gp, hp, psum)

        xt = const.tile([P, 3 * NXT], BF, name="xt")
        nc.sync.dma_start(xt[:], xT_d[:])
        bias = const.tile([P, 32], F32, name="bias_sb")
        nc.sync.dma_start(bias[:], bias_d[:])
        ident = const.tile([P, P], BF, name="ident_sb")
        nc.sync.dma_start(ident[:], ident_d[:])
        xres = const.tile([P, 32 * NSM], BF, name="xres")
        xtop_sb = const.tile([P, 32 * NTOP], BF, name="xtop_sb")
        xproj = dram.tile([P, 32 * 896], BF, name="xproj")  # mid levels 12..10

        h_ch, c_ch = _emit_xproj_and_leaf(nc, xt, bias, wxT_d, xproj, xres, xtop_sb, pools)
        nc.sync.dma_start(xtop_d[:], xtop_sb[:])

        # weights for the h-GEMMs load during the xproj/leaf pass
        wh = const.tile([P, 4 * MT * MEM], BF, name="wh_sb")
        for gi in range(4):
            s = gi * MT * MEM
            nc.sync.dma_start(wh[:, s : s + MT * MEM], whT_d[:, s : s + MT * MEM])

        for lvl in range(12, 2, -1):
            n = 1 << (lvl - 3)
            par = lvl & 1
            if lvl >= 10:
                off = OFF[lvl] - 1024

                def xp_of(gi, m, off=off, n=n, lvl=lvl):
                    xp = xpp.tile([P, n], BF, tag="xp", bufs=8, name=f"xpl_{lvl}_{gi}_{m}")
                    nc.sync.dma_start(
                        xp[:], xproj[:, (gi * MT + m) * 896 + off : (gi * MT + m) * 896 + off + n]
                    )
                    return xp[:]

            else:
                off = OFF[lvl] - SM0

                def xp_of(gi, m, off=off, n=n):
                    return xres[:, (gi * MT + m) * NSM + off : (gi * MT + m) * NSM + off + n]

            h_ch, c_ch = _emit_level(nc, wh, ident, h_ch, c_ch, n, xp_of, par, pools, lvl)

        out32 = gp.tile([P, 16], F32, tag="g", bufs=12, name="out32")
        nc.vector.tensor_copy(out32[:, 0:8], h_ch[:])
        nc.vector.tensor_copy(out32[:, 8:16], c_ch[:])
        nc.sync.dma_start(h3c3_d[:], out32[:])
    nc.compile()
    return nc


def build_stage_b():
    global FUSED_F
    FUSED_F = False  # at n<=4 the xpf2 copies cost more latency than 8 LDWs save
    nc = bacc.Bacc("TRN2", target_bir_lowering=False, debug=False, num_devices=1)
    h3_d = nc.dram_tensor("h3", [P, MT * 8], BF, kind="ExternalInput").ap()
    c3_d = nc.dram_tensor("c3", [P, MT * 8], BF, kind="ExternalInput").ap()
    whT_d = nc.dram_tensor("whT", [P, 4 * MT * MEM], BF, kind="ExternalInput").ap()
    xtop_d = nc.dram_tensor("xtop", [P, 32 * NTOP], BF, kind="ExternalInput").ap()
    ident_d = nc.dram_tensor("ident", [P, P], BF, kind="ExternalInput").ap()
    root_d = nc.dram_tensor("root", [P, 16], F32, kind="ExternalOutput").ap()

    TOFF = {2: 0, 1: 4, 0: 6}

    with tile.TileContext(nc) as tc, ExitStack() as ctx:
        const = ctx.enter_context(tc.tile_pool(name="const", bufs=1))
        xpp = ctx.enter_context(tc.tile_pool(name="xpp", bufs=8))
        gp = ctx.enter_context(tc.tile_pool(name="gp", bufs=12))
        hp = ctx.enter_context(tc.tile_pool(name="hp", bufs=1))
        psum = ctx.enter_context(tc.tile_pool(name="psum", bufs=8, space="PSUM"))
        pools = (xpp, gp, hp, psum)

        wh = const.tile([P, 4 * MT * MEM], BF, name="wh_sb")
        for gi in range(4):
            s = gi * MT * MEM
            nc.sync.dma_start(wh[:, s : s + MT * MEM], whT_d[:, s : s + MT * MEM])
        ident = const.tile([P, P], BF, name="ident_sb")
        nc.sync.dma_start(ident[:], ident_d[:])
        xtop = const.tile([P, 32 * NTOP], BF, name="xtop_sb")
        nc.sync.dma_start(xtop[:], xtop_d[:])

        h_ch = hp.tile([P, MT * 8], BF, tag="h_odd", name="h_in")
        nc.sync.dma_start(h_ch[:], h3_d[:])
        c_ch = hp.tile([P, MT * 8], BF, tag="c_odd", name="c_in")
        nc.sync.dma_start(c_ch[:], c3_d[:])

        for lvl in range(2, -1, -1):
            n = 1 << lvl
            toff = TOFF[lvl]
            par = lvl & 1

            def xp_of(gi, m, toff=toff, n=n):
                return xtop[:, (gi * MT + m) * NTOP + toff : (gi * MT + m) * NTOP + toff + n]

            h_ch, c_ch = _emit_level(nc, wh, ident, h_ch, c_ch, n, xp_of, par, pools, lvl)

        out32 = gp.tile([P, 16], F32, tag="g", bufs=12, name="out32")
        nc.vector.tensor_copy(out32[:, 0:8], c_ch[:])
        nc.vector.tensor_copy(out32[:, 8:16], h_ch[:])
        nc.sync.dma_start(root_d[:], out32[:])
    nc.compile()
    FUSED_F = True
    return nc


_CACHE = {}


def _get_programs():
    if "a" not in _CACHE:
        _CACHE["a"] = build_stage_a()
        _CACHE["b"] = build_stage_b()
    return _CACHE["a"], _CACHE["b"]


def _prep_host_inputs(embs, Ws, bs):
    wxT = np.zeros((P, 4 * 3 * MEM), BF16)
    whT = np.zeros((P, 4 * MT * MEM), BF16)
    bias = np.zeros((P, 32), np.float32)
    for gi, g in enumerate(GATES):
        WxT = Ws[g + "x"].T.astype(BF16)  # [300, 1024]
        for kx in range(3):
            rows = WxT[kx * P : (kx + 1) * P]
            wxT[: rows.shape[0], (gi * 3 + kx) * MEM : (gi * 3 + kx + 1) * MEM] = rows
        WhT = Ws[g + "h"].T.astype(BF16)  # [1024, 1024]
        for k in range(MT):
            whT[:, (gi * MT + k) * MEM : (gi * MT + k + 1) * MEM] = WhT[k * P : (k + 1) * P]
        bias[:, gi * MT : (gi + 1) * MT] = bs[g].reshape(MT, P).T
    ident = np.eye(P, dtype=BF16)

    x_top = embs[[3, 4, 5, 6, 1, 2, 0]].T  # [300, 7], heap order per level
    in_maps = []
    for k in range(NCORE):
        cols = []
        for lvl in range(DEPTH - 1, 2, -1):
            n = 1 << lvl
            nl = n // NCORE
            cols.append(embs[n - 1 + k * nl : n - 1 + (k + 1) * nl].T)
        x_all = np.concatenate(cols + [x_top], axis=1)  # [300, 2054]
        xT = np.zeros((P, 3 * NXT), BF16)
        for kx in range(3):
            rows = x_all[kx * P : (kx + 1) * P].astype(BF16)
            xT[: rows.shape[0], kx * NXT : (kx + 1) * NXT] = rows
        in_maps.append({"xT": xT, "wxT": wxT, "whT": whT, "bias": bias, "ident": ident})
    return in_maps, whT, ident


def kernel(**inputs):
    embs = np.asarray(inputs["embs"], dtype=np.float32)
    depth = int(np.asarray(inputs["depth"]))
    assert depth == DEPTH and embs.shape == (2**DEPTH - 1, IN)
    Ws = {g + s: np.asarray(inputs["W" + g + s], dtype=np.float32) for g in GATES for s in "xh"}
    bs = {g: np.asarray(inputs["b" + g + "x"]) + np.asarray(inputs["b" + g + "h"]) for g in GATES}

    nc_a, nc_b = _get_programs()
    in_maps, whT, ident = _prep_host_inputs(embs, Ws, bs)
    res_a = run_bass_kernel_spmd(nc_a, in_maps, core_ids=list(range(NCORE))).results

    h3 = np.zeros((P, MT * 8), BF16)
    c3 = np.zeros((P, MT * 8), BF16)
    for j in range(NCORE):
        hc = res_a[j]["h3c3"]  # [128, 16] f32
        for m in range(MT):
            h3[:, m * 8 + j] = hc[:, m].astype(BF16)
            c3[:, m * 8 + j] = hc[:, 8 + m].astype(BF16)
    xtop = res_a[0]["xtop"]

    in_b = {"h3": h3, "c3": c3, "whT": whT, "xtop": np.asarray(xtop, dtype=BF16), "ident": ident}
    res_b = run_bass_kernel_spmd(nc_b, [in_b], core_ids=[0]).results
    root = res_b[0]["root"]  # [128, 16] f32: cols 0..7 = c M-tiles, 8..15 = h
    c_root = root[:, :8].T.reshape(MEM)
    h_root = root[:, 8:].T.reshape(MEM)
    return np.stack([c_root, h_root]).astype(np.float32)
